# revision 1
# baseline (speedup 1.0000x reference)
"""Trainium2 Bass kernel for nn_NodeSemanticAndStructureModel.

Model (reference):
  h_sem = leaky(x @ W_sem + b_sem)           [N, H]
  h_str = leaky(x_struct @ W_str + b_str)    [N, H]
  h     = BN1(concat(h_sem, h_str))          [N, 2H]   (batch stats over N)
  h2    = BN2(tanh(h @ Wf + bf))             [N, H]
  agg   = segment_min(h2[src], dst, N); empty -> 0
  out   = relu(agg @ Wc1 + bc1) @ Wc2 + bc2  [N, OUT]

Distribution (8 cores): nodes are sharded (6250/core); edges are partitioned
by destination shard.  Each core computes h2 for its nodes in a *degree
sorted* order (sorted by local in-degree, descending), all cores AllGather
the h2 table, and each core then computes the segment-min for its own
destinations via indirect-DMA gathers in "rounds": node-tile t (128 nodes on
partitions) round k gathers the k-th edge of every node in the tile; a DVE
min-reduce folds the rounds.  Degree sorting makes the per-tile round count
tight (total gathered rows ~= E/8 + a few %).

BN trickery: BN1's scale/shift is folded into Wf/bf (weights are adjusted on
device after a tiny AllReduce of the batch moments).  BN2 is applied *after*
aggregation: the table stores sign(gamma2) * tanh(...), so
min(a2*t + b2) == |a2| * min(sign(a2)*t) + b2, and |a2|/b2 are folded into
Wc1/bc1.  This keeps the BN2 AllReduce completely off the critical path.

Everything runs in a transposed activation layout ([features on partitions,
nodes on free]) so matmuls contract over the partition dim natively; the two
places that need node-major data (the h2 table, the aggregated features) use
PE transposes.
"""

import math
import numpy as np

import concourse.bass as bass
import concourse.tile as tile
from concourse import mybir
from concourse.bass import IndirectOffsetOnAxis
from concourse.bass_utils import run_bass_kernel_spmd
from concourse.masks import make_identity
from concourse.tile import add_dep_helper

F32 = mybir.dt.float32
F32R = mybir.dt.float32r
I32 = mybir.dt.int32

# problem dims (hardcoded per contract)
C = 8
N = 50000
NS = N // C           # 6250 nodes per core
IN = 1024
STR = 768
H = 256
H2 = 2 * H            # 512
OUT = 64
EPS = 1e-5

KI = IN // 128        # 8
KS = STR // 128       # 6
HC = H // 128         # 2
K2 = H2 // 128        # 4

FT = 512              # free-dim node tile for phases A/B
NT = (NS + 127) // 128   # 49 node tiles for the aggregation phase
PAD = NT * 128           # 6272
RMAX = 16             # max gather rounds folded into one indirect DMA

VE = 25               # packed small-vector columns
LINEARIZE = False


def _r(ap):
    return ap.bitcast(F32R)


def _col_tiles(n, t):
    out = []
    o = 0
    while o < n:
        out.append((o, min(t, n - o)))
        o += t
    return out


def build_program(schedule, total_r):
    """Build the SPMD Bass program.  `schedule` is a list (len NT) of lists of
    chunk sizes (each <= RMAX); identical on every core.

    Wait-budget discipline: a self-loading fp32r Matmult can carry at most ONE
    sync wait in codegen, i.e. it may depend on at most one "proc" (engine /
    DMA lane) whose semaphore tick the PE has not already observed.  So every
    tensor a matmul reads is last-written by ACT (phases A/B) and DMA waits
    are absorbed by PE nops (pinned before their matmul group with non-sync
    edges).  Phase C reductions run on DVE; a per-group PE nop observes the
    DVE tick before the transposes/classifier matmuls run.
    """
    nc = bass.Bass()
    AF = mybir.ActivationFunctionType

    xT = nc.declare_dram_parameter("xT", [IN, NS], F32R, isOutput=False)
    xsT = nc.declare_dram_parameter("xsT", [STR, NS], F32R, isOutput=False)
    idxd = nc.declare_dram_parameter("idx", [128, total_r], I32, isOutput=False)
    wsem = nc.declare_dram_parameter("wsem", [IN, H], F32R, isOutput=False)
    wstr = nc.declare_dram_parameter("wstr", [STR, H], F32R, isOutput=False)
    wf = nc.declare_dram_parameter("wf", [H2, H], F32R, isOutput=False)
    wc1 = nc.declare_dram_parameter("wc1", [H, H], F32R, isOutput=False)
    wc2 = nc.declare_dram_parameter("wc2", [H, OUT], F32R, isOutput=False)
    vecs = nc.declare_dram_parameter("vecs", [128, VE], F32, isOutput=False)
    outT = nc.declare_dram_parameter("outT", [OUT, PAD], F32, isOutput=True)

    table_local = nc.dram_tensor("table_local", [NS, H], F32)
    table = nc.dram_tensor("table", [C * NS, H], F32, addr_space="Shared")
    bn1_in = nc.dram_tensor("bn1_in", [128, 8], F32)
    bn1_out = nc.dram_tensor("bn1_out", [128, 8], F32, addr_space="Shared")
    bn2_in = nc.dram_tensor("bn2_in", [128, 4], F32)
    bn2_out = nc.dram_tensor("bn2_out", [128, 4], F32, addr_space="Shared")

    RG = [list(range(C))]
    ntiles = _col_tiles(NS, FT)
    n_ft = len(ntiles)

    with tile.TileContext(nc, linearize=LINEARIZE) as tc:
        touch_state = {}

        def pe_touch(ap):
            """Tiny matmul reading `ap` so the PE's vector clock observes the
            producer's semaphore tick via a REAL data dep (a 1-wait
            instruction); later matmuls reading the same producer then carry
            no extra wait.  Output goes to one persistent write-only psum
            (same tile every time -> same-engine WAW, no slot-release sems)."""
            if "pt" not in touch_state:
                ptile = touch_state["pool"].tile([1, 1], F32, tag="touch")
                touch_state["pt"] = ptile
            apf = ap.bitcast(F32) if ap.dtype == F32R else ap
            mm = nc.tensor.matmul(touch_state["pt"][:], apf, apf,
                                  start=True, stop=True)
            return mm

        def dve_touch(ap):
            """Tiny DVE op reading `ap` (same trick for the vector engine)."""
            ts = touch_state["sc"]
            return nc.vector.tensor_scalar_mul(out=ts[:], in0=ap, scalar1=1.0)

        def pin_after(mm, nop):
            if nop is not None:
                add_dep_helper(mm.ins, nop.ins, sync=False, reason="pe-order")

        with (
            tc.tile_pool(name="const", bufs=1) as cp,
            tc.tile_pool(name="psA", bufs=3, space="PSUM") as psA,
            tc.tile_pool(name="psT", bufs=2, space="PSUM") as psT,
            tc.tile_pool(name="psV", bufs=2, space="PSUM") as psV,
            tc.tile_pool(name="tp", bufs=1, space="PSUM") as tpool,
        ):
            touch_state["pool"] = tpool
            dvesc = cp.tile([128, 1], F32, tag="dvesc")
            touch_state["sc"] = dvesc
            # ---- constants ----
            ident = cp.tile([128, 128], F32, tag="ident")
            make_identity(nc, ident[:])
            ws_sb = cp.tile([128, KI, H], F32R, tag="ws")
            d1 = nc.sync.dma_start(out=ws_sb[:], in_=wsem[:].rearrange("(k p) h -> p k h", p=128))
            wsr_sb = cp.tile([128, KS, H], F32R, tag="wsr")
            d2 = nc.sync.dma_start(out=wsr_sb[:], in_=wstr[:].rearrange("(k p) h -> p k h", p=128))
            wf_sb = cp.tile([128, K2, H], F32R, tag="wfs")
            d3 = nc.sync.dma_start(out=wf_sb[:], in_=wf[:].rearrange("(k p) h -> p k h", p=128))
            wc1_sb = cp.tile([128, HC, H], F32R, tag="wc1s")
            d4 = nc.sync.dma_start(out=wc1_sb[:], in_=wc1[:].rearrange("(k p) h -> p k h", p=128))
            wc2_sb = cp.tile([128, HC, OUT], F32R, tag="wc2s")
            d5 = nc.sync.dma_start(out=wc2_sb[:], in_=wc2[:].rearrange("(k p) o -> p k o", p=128))
            vec_sb = cp.tile([128, VE], F32, tag="vecs")
            d6 = nc.sync.dma_start(out=vec_sb[:], in_=vecs[:])
            pe_touch(ident[:, 0:1])
            pe_touch(ws_sb[:, 0, 0:1])
            pe_touch(wsr_sb[:, 0, 0:1])
            pe_touch(wf_sb[:, 0, 0:1])
            pe_touch(wc1_sb[:, 0, 0:1])
            cnop = pe_touch(wc2_sb[:, 0, 0:1])
            # ACT / DVE observe the vec DMA lane once, so later bias/scale
            # reads never add a DMA wait to compute instructions.
            vtouch = cp.tile([128, 1], F32, tag="vt")
            vtouch2 = cp.tile([128, 1], F32, tag="vt2")
            nc.scalar.activation(out=vtouch[:], in_=vec_sb[:, 0:1], func=AF.Copy)
            nc.vector.tensor_scalar_mul(out=vtouch2[:], in0=vec_sb[:, 0:1],
                                        scalar1=1.0)

            # packed columns
            b_sem = vec_sb[:, 0:2]
            b_str = vec_sb[:, 2:4]
            gam1 = vec_sb[:, 4:8]
            bet1 = vec_sb[:, 8:12]
            bf_c = vec_sb[:, 12:14]
            gam2 = vec_sb[:, 14:16]
            bet2 = vec_sb[:, 16:18]
            bc1_c = vec_sb[:, 18:20]
            sflip = vec_sb[:, 20:22]
            bc2_c = vec_sb[:, 22:23]
            eps_c = vec_sb[:, 23:24]

            sums1 = cp.tile([128, K2, n_ft], F32, tag="sums1")
            sqs1 = cp.tile([128, K2, n_ft], F32, tag="sqs1")
            sums2 = cp.tile([128, HC, n_ft], F32, tag="sums2")
            sqs2 = cp.tile([128, HC, n_ft], F32, tag="sqs2")
            biasF = cp.tile([128, HC], F32, tag="biasF")
            bias1 = cp.tile([128, HC], F32, tag="bias1")

            last_asm = [None]
            last_tanh = [None]

            # ================= phase A: refiners =================
            with (
                tc.tile_pool(name="hp", bufs=1) as hp,
                tc.tile_pool(name="xp", bufs=2) as xp,
                tc.tile_pool(name="t2p", bufs=4) as t2p,
                tc.tile_pool(name="asmp", bufs=3) as asmp,
            ):
                hT = hp.tile([128, K2, NS], F32R, tag="hT")

                def refiner(src_ap, w_sb, nk, bias_c, fc0, n0, nsz, nti, nop):
                    for hc in range(HC):
                        ps = psA.tile([128, nsz], F32, tag="mm")
                        for k in range(nk):
                            mm = nc.tensor.matmul(
                                ps[:], w_sb[:, k, hc * 128:(hc + 1) * 128],
                                src_ap[:, k, :], start=(k == 0), stop=(k == nk - 1))
                            if k == 0:
                                pin_after(mm, nop)
                        lin = t2p.tile([128, nsz], F32, tag="lk0")
                        nc.scalar.activation(out=lin[:], in_=ps[:], func=AF.Identity,
                                             bias=bias_c[:, hc:hc + 1], scale=1.0)
                        tmp = t2p.tile([128, nsz], F32, tag="lk1")
                        nc.scalar.mul(out=tmp[:], in_=lin[:], mul=0.01)
                        lk2 = t2p.tile([128, nsz], F32, tag="lk2")
                        nc.vector.tensor_tensor(out=lk2[:], in0=lin[:], in1=tmp[:],
                                                op=mybir.AluOpType.max)
                        hdst = hT[:, fc0 + hc, n0:n0 + nsz]
                        nc.scalar.activation(out=hdst, in_=lk2[:], func=AF.Identity,
                                             bias=0.0, scale=1.0)
                        nc.vector.tensor_reduce(
                            out=sums1[:, fc0 + hc, nti:nti + 1], in_=lk2[:],
                            op=mybir.AluOpType.add, axis=mybir.AxisListType.X)
                        sq = t2p.tile([128, nsz], F32, tag="sq")
                        nc.scalar.activation(out=sq[:], in_=lk2[:], func=AF.Square)
                        nc.vector.tensor_reduce(
                            out=sqs1[:, fc0 + hc, nti:nti + 1], in_=sq[:],
                            op=mybir.AluOpType.add, axis=mybir.AxisListType.X)

                for nti, (n0, nsz) in enumerate(ntiles):
                    xk = xp.tile([128, KI, nsz], F32R, tag="xin")
                    nc.sync.dma_start(
                        out=xk[:],
                        in_=xT[:].rearrange("(k p) n -> p k n", p=128)[:, :, n0:n0 + nsz])
                    nopx = pe_touch(xk[:, 0, 0:1])
                    refiner(xk, ws_sb, KI, b_sem, 0, n0, nsz, nti, nopx)
                    xs = xp.tile([128, KS, nsz], F32R, tag="xin")
                    nc.sync.dma_start(
                        out=xs[:],
                        in_=xsT[:].rearrange("(k p) n -> p k n", p=128)[:, :, n0:n0 + nsz])
                    nops = pe_touch(xs[:, 0, 0:1])
                    refiner(xs, wsr_sb, KS, b_str, HC, n0, nsz, nti, nops)

                # ---- BN1 moments -> AllReduce -> fold into Wf ----
                pay1 = cp.tile([128, 8], F32, tag="pay1")
                for fc in range(K2):
                    nc.vector.tensor_reduce(
                        out=pay1[:, fc:fc + 1], in_=sums1[:, fc, :],
                        op=mybir.AluOpType.add, axis=mybir.AxisListType.X)
                    nc.vector.tensor_reduce(
                        out=pay1[:, 4 + fc:5 + fc], in_=sqs1[:, fc, :],
                        op=mybir.AluOpType.add, axis=mybir.AxisListType.X)
                nc.gpsimd.dma_start(out=bn1_in[:], in_=pay1[:])
                nc.gpsimd.collective_compute(
                    "AllReduce", mybir.AluOpType.add, ins=[bn1_in[:]], outs=[bn1_out[:]],
                    replica_groups=RG)
                red1 = cp.tile([128, 8], F32, tag="red1")
                rd1 = nc.gpsimd.dma_start(out=red1[:], in_=bn1_out[:])
                mg = cp.tile([128, K2], F32, tag="mg1")
                a1 = cp.tile([128, K2], F32, tag="a1")
                b1f = cp.tile([128, K2], F32, tag="b1f")
                b1 = cp.tile([128, K2], F32R, tag="b1")
                nc.vector.tensor_scalar_mul(out=mg[:], in0=red1[:, 0:4],
                                            scalar1=1.0 / (C * NS))
                nc.vector.tensor_scalar_mul(out=a1[:], in0=red1[:, 4:8],
                                            scalar1=1.0 / (C * NS))
                nc.vector.tensor_tensor(out=b1f[:], in0=mg[:], in1=mg[:],
                                        op=mybir.AluOpType.mult)
                nc.vector.tensor_tensor(out=a1[:], in0=a1[:], in1=b1f[:],
                                        op=mybir.AluOpType.subtract)
                nc.scalar.activation(out=a1[:], in_=a1[:], func=AF.Sqrt,
                                     bias=eps_c, scale=1.0)
                nc.vector.reciprocal(out=a1[:], in_=a1[:])
                nc.vector.tensor_tensor(out=a1[:], in0=a1[:], in1=gam1,
                                        op=mybir.AluOpType.mult)
                nc.vector.tensor_tensor(out=b1f[:], in0=mg[:], in1=a1[:],
                                        op=mybir.AluOpType.mult)
                nc.vector.tensor_tensor(out=b1f[:], in0=bet1, in1=b1f[:],
                                        op=mybir.AluOpType.subtract)
                nc.scalar.activation(out=b1[:], in_=b1f[:], func=AF.Identity)
                # biasF = b1 @ Wf + bf (original Wf), then scale Wf rows by a1
                for hc in range(HC):
                    pv = psV.tile([128, 1], F32, tag="v")
                    for k in range(K2):
                        nc.tensor.matmul(pv[:],
                                         wf_sb[:, k, hc * 128:(hc + 1) * 128].bitcast(F32),
                                         b1[:, k:k + 1].bitcast(F32), start=(k == 0),
                                         stop=(k == K2 - 1))
                    nc.scalar.activation(out=biasF[:, hc:hc + 1], in_=pv[:],
                                         func=AF.Identity,
                                         bias=bf_c[:, hc:hc + 1], scale=1.0)
                for k in range(K2):
                    nc.scalar.activation(out=wf_sb[:, k, :],
                                         in_=wf_sb[:, k, :].bitcast(F32),
                                         func=AF.Identity, bias=0.0,
                                         scale=a1[:, k:k + 1])

                # ================= phase B: fusion + table =================
                for nti, (n0, nsz) in enumerate(ntiles):
                    t2s = []
                    for hc in range(HC):
                        ps = psA.tile([128, nsz], F32, tag="mm")
                        for k in range(K2):
                            nc.tensor.matmul(
                                ps[:], wf_sb[:, k, hc * 128:(hc + 1) * 128],
                                hT[:, k, n0:n0 + nsz], start=(k == 0),
                                stop=(k == K2 - 1))
                        t2 = t2p.tile([128, nsz], F32, tag="t2")
                        tan = nc.scalar.activation(out=t2[:], in_=ps[:], func=AF.Tanh,
                                                   bias=biasF[:, hc:hc + 1], scale=1.0)
                        last_tanh[0] = tan
                        nc.vector.tensor_reduce(
                            out=sums2[:, hc, nti:nti + 1], in_=t2[:],
                            op=mybir.AluOpType.add, axis=mybir.AxisListType.X)
                        sq = t2p.tile([128, nsz], F32, tag="sq")
                        nc.scalar.activation(out=sq[:], in_=t2[:], func=AF.Square)
                        nc.vector.tensor_reduce(
                            out=sqs2[:, hc, nti:nti + 1], in_=sq[:],
                            op=mybir.AluOpType.add, axis=mybir.AxisListType.X)
                        ts = t2p.tile([128, nsz], F32, tag="t2s")
                        nc.scalar.activation(out=ts[:], in_=t2[:], func=AF.Identity,
                                             bias=0.0, scale=sflip[:, hc:hc + 1])
                        t2s.append(ts)
                    for nb in range((nsz + 127) // 128):
                        bsz = min(128, nsz - nb * 128)
                        asm = asmp.tile([128, HC, 128], F32, tag="asm")
                        for hc in range(HC):
                            pt = psT.tile([128, 128], F32, tag="tr")
                            nc.tensor.transpose(
                                pt[:bsz, :], t2s[hc][:, nb * 128:nb * 128 + bsz], ident[:])
                            ac = nc.scalar.activation(out=asm[:bsz, hc, :],
                                                      in_=pt[:bsz, :], func=AF.Copy)
                            last_asm[0] = ac
                        r0 = n0 + nb * 128
                        nc.sync.dma_start(
                            out=table_local[r0:r0 + bsz, :].rearrange(
                                "n (a b) -> n a b", a=HC),
                            in_=asm[:bsz, :, :])

            # ---- collectives: table AllGather + BN2 AllReduce ----
            nc.gpsimd.collective_compute(
                "AllGather", mybir.AluOpType.bypass, ins=[table_local[:]],
                outs=[table[:]], replica_groups=RG)

            pay2 = cp.tile([128, 4], F32, tag="pay2")
            for hc in range(HC):
                nc.vector.tensor_reduce(
                    out=pay2[:, hc:hc + 1], in_=sums2[:, hc, :],
                    op=mybir.AluOpType.add, axis=mybir.AxisListType.X)
                nc.vector.tensor_reduce(
                    out=pay2[:, 2 + hc:3 + hc], in_=sqs2[:, hc, :],
                    op=mybir.AluOpType.add, axis=mybir.AxisListType.X)
            nc.gpsimd.dma_start(out=bn2_in[:], in_=pay2[:])
            nc.gpsimd.collective_compute(
                "AllReduce", mybir.AluOpType.add, ins=[bn2_in[:]], outs=[bn2_out[:]],
                replica_groups=RG)
            red2 = cp.tile([128, 4], F32, tag="red2")
            nc.gpsimd.dma_start(out=red2[:], in_=bn2_out[:])
            mg2 = cp.tile([128, HC], F32, tag="mg2")
            a2 = cp.tile([128, HC], F32, tag="a2")   # gamma2*rstd (signed)
            b2f = cp.tile([128, HC], F32, tag="b2f")
            b2 = cp.tile([128, HC], F32R, tag="b2")
            nc.vector.tensor_scalar_mul(out=mg2[:], in0=red2[:, 0:2],
                                        scalar1=1.0 / (C * NS))
            nc.vector.tensor_scalar_mul(out=a2[:], in0=red2[:, 2:4],
                                        scalar1=1.0 / (C * NS))
            nc.vector.tensor_tensor(out=b2f[:], in0=mg2[:], in1=mg2[:],
                                    op=mybir.AluOpType.mult)
            nc.vector.tensor_tensor(out=a2[:], in0=a2[:], in1=b2f[:],
                                    op=mybir.AluOpType.subtract)
            nc.scalar.activation(out=a2[:], in_=a2[:], func=AF.Sqrt,
                                 bias=eps_c, scale=1.0)
            nc.vector.reciprocal(out=a2[:], in_=a2[:])
            nc.vector.tensor_tensor(out=a2[:], in0=a2[:], in1=gam2,
                                    op=mybir.AluOpType.mult)
            nc.vector.tensor_tensor(out=b2f[:], in0=mg2[:], in1=a2[:],
                                    op=mybir.AluOpType.mult)
            nc.vector.tensor_tensor(out=b2f[:], in0=bet2, in1=b2f[:],
                                    op=mybir.AluOpType.subtract)
            nc.scalar.activation(out=b2[:], in_=b2f[:], func=AF.Identity)
            # bias1 = b2 @ Wc1 + bc1 (original Wc1); then Wc1 rows *= |a2|
            for hc in range(HC):
                pv = psV.tile([128, 1], F32, tag="v")
                for k in range(HC):
                    nc.tensor.matmul(pv[:],
                                     wc1_sb[:, k, hc * 128:(hc + 1) * 128].bitcast(F32),
                                     b2[:, k:k + 1].bitcast(F32), start=(k == 0),
                                     stop=(k == HC - 1))
                nc.scalar.activation(out=bias1[:, hc:hc + 1], in_=pv[:],
                                     func=AF.Identity,
                                     bias=bc1_c[:, hc:hc + 1], scale=1.0)
            a2a = cp.tile([128, HC], F32, tag="a2a")
            nc.vector.tensor_scalar_mul(out=a2a[:], in0=a2[:], scalar1=-1.0)
            nc.vector.tensor_tensor(out=a2a[:], in0=a2a[:], in1=a2[:],
                                    op=mybir.AluOpType.max)
            for k in range(HC):
                nc.scalar.activation(out=wc1_sb[:, k, :],
                                     in_=wc1_sb[:, k, :].bitcast(F32),
                                     func=AF.Identity, bias=0.0,
                                     scale=a2a[:, k:k + 1])

            # ================= phase C: gather-min + classifier =================
            with (
                tc.tile_pool(name="idxp", bufs=1) as idxp,
                tc.tile_pool(name="gp", bufs=8) as gp,
                tc.tile_pool(name="accp", bufs=6) as accp,
                tc.tile_pool(name="redp", bufs=3) as redp,
                tc.tile_pool(name="aggp", bufs=2) as aggp,
                tc.tile_pool(name="r1p", bufs=2) as r1p,
                tc.tile_pool(name="otp", bufs=3) as otp,
            ):
                idx_sb = idxp.tile([128, total_r], I32, tag="idx")
                idma = nc.gpsimd.dma_start(out=idx_sb[:], in_=idxd[:])
                offs = np.cumsum([0] + [sum(s) for s in schedule]).tolist()
                # absorb the conservative block-entry PE wait Tile emits on
                # the first PE instruction after the phase-B pools close
                # (anchored in this region via a dep on the idx DMA)
                c_nop = nc.tensor.nop()
                add_dep_helper(c_nop.ins, idma.ins, sync=True, reason="anchor")

                GRP = 4
                for g0 in range(0, NT, GRP):
                    tl = list(range(g0, min(g0 + GRP, NT)))
                    gsz = len(tl) * 128
                    aggT = aggp.tile([128, HC, gsz], F32R, tag="aggT")
                    accs = []
                    for ti, t in enumerate(tl):
                        acc = accp.tile([128, H], F32, tag="acc")
                        off = offs[t]
                        for j, csz in enumerate(schedule[t]):
                            gb = gp.tile([128, H], F32, tag="gb")
                            nc.gpsimd.indirect_dma_start(
                                out=gb[:], out_offset=None, in_=table[:],
                                in_offset=IndirectOffsetOnAxis(
                                    ap=idx_sb[:, off:off + 1], axis=0),
                            )
                            if j == 0:
                                nc.vector.tensor_copy(out=acc[:], in_=gb[:])
                            else:
                                nc.vector.tensor_tensor(
                                    out=acc[:], in0=acc[:], in1=gb[:],
                                    op=mybir.AluOpType.min)
                            off += csz
                        accs.append(acc)
                    gnop = None
                    for a in accs:
                        gnop = pe_touch(a[:, 0:1])
                        if g0 == 0:
                            add_dep_helper(gnop.ins, c_nop.ins, sync=False,
                                           reason="pe-order")
                    for ti, t in enumerate(tl):
                        for fc in range(HC):
                            pt = psT.tile([128, 128], F32, tag="tr")
                            tr = nc.tensor.transpose(
                                pt[:], accs[ti][:, fc * 128:(fc + 1) * 128], ident[:])
                            pin_after(tr, gnop)
                            nc.scalar.activation(
                                out=aggT[:, fc, ti * 128:(ti + 1) * 128], in_=pt[:],
                                func=AF.Copy)
                    r1 = r1p.tile([128, HC, gsz], F32R, tag="r1")
                    for hc in range(HC):
                        ps = psA.tile([128, gsz], F32, tag="mm")
                        for k in range(HC):
                            mm = nc.tensor.matmul(
                                ps[:], wc1_sb[:, k, hc * 128:(hc + 1) * 128],
                                aggT[:, k, :], start=(k == 0), stop=(k == HC - 1))
                            if k == 0:
                                pin_after(mm, gnop)
                        nc.scalar.activation(out=r1[:, hc, :], in_=ps[:], func=AF.Relu,
                                             bias=bias1[:, hc:hc + 1], scale=1.0)
                    ps2 = psA.tile([64, gsz], F32, tag="mm")
                    for k in range(HC):
                        nc.tensor.matmul(ps2[:], wc2_sb[:, k, :], r1[:, k, :],
                                         start=(k == 0), stop=(k == HC - 1))
                    ot = otp.tile([64, gsz], F32, tag="ot")
                    nc.scalar.activation(out=ot[:], in_=ps2[:], func=AF.Identity,
                                         bias=bc2_c[:64, :], scale=1.0)
                    nc.sync.dma_start(out=outT[:, g0 * 128:g0 * 128 + gsz], in_=ot[:])
                with nc.allow_non_contiguous_dma(reason="debug"):
                    nc.sync.dma_start(out=outT[:, NS + 21:NS + 22],
                                      in_=table[0:64, 0:1])
                    nc.sync.dma_start(out=outT[:, NS + 20:NS + 21],
                                      in_=table_local[0:64, 0:1])
                # debug: dump BN intermediates into unused padded columns
                nc.sync.dma_start(out=outT[:, NS:NS + 4], in_=a1[:64, :])
                nc.sync.dma_start(out=outT[:, NS + 4:NS + 6], in_=a2[:64, :])
                nc.sync.dma_start(out=outT[:, NS + 6:NS + 14], in_=red1[:64, :])
                nc.sync.dma_start(out=outT[:, NS + 14:NS + 18], in_=red2[:64, :])

    return nc


def _split_excess_waits(nc, budget=1):
    """Walrus codegen in this container rejects instructions carrying more
    than one sync wait.  Move excess waits onto standalone EventSemaphore
    instructions inserted immediately before the offender on the same
    engine queue (the same mechanism Tile's own barriers use)."""
    n = 0
    for f in nc.m.functions:
        for bb in f.blocks:
            out = []
            for ins in bb.instructions:
                si = ins.sync_info
                waits = list(si.on_wait) if si and si.on_wait else []
                if len(waits) > budget:
                    for w in waits[:-budget]:
                        ev = mybir.InstEventSemaphore(
                            name=f"evw-{n}", ins=[], outs=[])
                        n += 1
                        ev.engine = ins.engine
                        ev.sync_info = mybir.SyncInfo(on_wait=[w], on_update=[])
                        out.append(ev)
                    si.on_wait = waits[-budget:]
                out.append(ins)
            bb.instructions = out
    return n


# ---------------------------------------------------------------------------
# host side
# ---------------------------------------------------------------------------

def _prep(edge_index):
    """Shard edges by destination, degree-sort nodes per shard, build the
    (shared) gather schedule and per-core index tables."""
    src = np.asarray(edge_index[0], dtype=np.int64)
    dst = np.asarray(edge_index[1], dtype=np.int64)
    owner = dst // NS
    dloc = (dst - owner * NS).astype(np.int64)

    deg = np.zeros((C, NS), np.int64)
    perm = np.zeros((C, NS), np.int64)
    rank = np.zeros((C, NS), np.int64)
    for r in range(C):
        m = owner == r
        deg[r] = np.bincount(dloc[m], minlength=NS)
        perm[r] = np.argsort(-deg[r], kind="stable")
        rank[r][perm[r]] = np.arange(NS)

    trow = np.empty(N, np.int64)
    for r in range(C):
        trow[r * NS:(r + 1) * NS] = r * NS + rank[r]

    sdeg = np.take_along_axis(deg, perm, axis=1)      # degrees in sorted order
    # shared schedule: per tile, number of rounds = max over cores
    d_t = []
    for t in range(NT):
        i0 = t * 128
        d = int(sdeg[:, i0].max()) if i0 < NS else 0
        d_t.append(max(d, 1))
    # HW indirect DMA supports exactly one offset per partition per
    # instruction, so every round is its own gather
    schedule = [[1] * d for d in d_t]
    total_r = sum(d_t)

    idx = np.zeros((C, 128, total_r), np.int32)
    dmax = max(d_t)
    for r in range(C):
        m = owner == r
        er = rank[r][dloc[m]]
        es = trow[src[m]]
        order = np.argsort(er, kind="stable")
        er = er[order]
        es = es[order]
        cum = np.concatenate([[0], np.cumsum(np.bincount(er, minlength=NS))])
        within = np.arange(len(er)) - cum[er]
        M = np.zeros((PAD, dmax), np.int64)
        fill = np.zeros(NS, np.int64)
        nz = sdeg[r] > 0
        fill[nz] = es[cum[:NS][nz]]
        M[:NS] = fill[:, None]
        M[er, within] = es
        o = 0
        for t in range(NT):
            d = d_t[t]
            idx[r, :, o:o + d] = M[t * 128:(t + 1) * 128, :d]
            o += d

    return deg, perm, schedule, total_r, idx


_CACHE = {}


def kernel(**inputs):
    x = np.asarray(inputs["x"], np.float32)
    xs = np.asarray(inputs["x_struct"], np.float32)
    ei = np.asarray(inputs["edge_index"])
    W_sem = np.asarray(inputs["W_sem"], np.float32)
    b_sem = np.asarray(inputs["b_sem"], np.float32)
    W_str = np.asarray(inputs["W_str"], np.float32)
    b_str = np.asarray(inputs["b_str"], np.float32)
    g1 = np.asarray(inputs["bn1_gamma"], np.float32)
    be1 = np.asarray(inputs["bn1_beta"], np.float32)
    Wf = np.asarray(inputs["Wf"], np.float32)
    bf = np.asarray(inputs["bf"], np.float32)
    g2 = np.asarray(inputs["bn2_gamma"], np.float32)
    be2 = np.asarray(inputs["bn2_beta"], np.float32)
    Wc1 = np.asarray(inputs["Wc1"], np.float32)
    bc1 = np.asarray(inputs["bc1"], np.float32)
    Wc2 = np.asarray(inputs["Wc2"], np.float32)
    bc2 = np.asarray(inputs["bc2"], np.float32)

    deg, perm, schedule, total_r, idx = _prep(ei)

    key = tuple(tuple(s) for s in schedule)
    if key not in _CACHE:
        prog = build_program(schedule, total_r)
        _split_excess_waits(prog)
        _CACHE[key] = prog
    nc = _CACHE[key]

    def pk2(v):   # [2*128] -> [128, 2] chunk-major
        return np.ascontiguousarray(v.reshape(-1, 128).T)

    vecs = np.zeros((128, VE), np.float32)
    vecs[:, 0:2] = pk2(b_sem)
    vecs[:, 2:4] = pk2(b_str)
    vecs[:, 4:8] = pk2(g1)
    vecs[:, 8:12] = pk2(be1)
    vecs[:, 12:14] = pk2(bf)
    vecs[:, 14:16] = pk2(g2)
    vecs[:, 16:18] = pk2(be2)
    vecs[:, 18:20] = pk2(bc1)
    vecs[:, 20:22] = pk2(np.where(g2 >= 0, 1.0, -1.0).astype(np.float32))
    vecs[:OUT, 22] = bc2
    vecs[:, 23] = EPS

    in_maps = []
    for r in range(C):
        gsel = r * NS + perm[r]
        in_maps.append({
            "xT": np.ascontiguousarray(x[gsel].T),
            "xsT": np.ascontiguousarray(xs[gsel].T),
            "idx": np.ascontiguousarray(idx[r]),
            "wsem": W_sem, "wstr": W_str, "wf": Wf, "wc1": Wc1, "wc2": Wc2,
            "vecs": vecs,
        })

    global _last_in_maps
    _last_in_maps = in_maps
    res = run_bass_kernel_spmd(nc, in_maps, list(range(C)))
    out = np.empty((N, OUT), np.float32)
    for r in range(C):
        oT = res.results[r]["outT"]
        out[r * NS + perm[r]] = oT[:, :NS].T

    # nodes with no incoming edges: reference yields relu(bc1) @ Wc2 + bc2
    # deg is indexed [core, local]; global id = core*NS + local
    empty = np.where(deg.reshape(-1) == 0)[0]
    if len(empty):
        const_row = np.maximum(bc1, 0.0) @ Wc2 + bc2
        out[empty] = const_row.astype(np.float32)
    return out



# revision 5
# speedup vs baseline: 16410.6665x; 16410.6665x over previous
"""Trainium2 Bass kernel for nn_NodeSemanticAndStructureModel.

Model (reference):
  h_sem = leaky(x @ W_sem + b_sem)           [N, H]
  h_str = leaky(x_struct @ W_str + b_str)    [N, H]
  h     = BN1(concat(h_sem, h_str))          [N, 2H]   (batch stats over N)
  h2    = BN2(tanh(h @ Wf + bf))             [N, H]
  agg   = segment_min(h2[src], dst, N); empty -> 0
  out   = relu(agg @ Wc1 + bc1) @ Wc2 + bc2  [N, OUT]

Distribution (8 cores): nodes are sharded (6250/core); edges are partitioned
by destination shard.  Each core computes h2 for its nodes (natural order),
all cores AllGather the h2 table (bf16), and each core computes the
segment-min for its own destinations with batched GPSIMD dma_gather
instructions: destination nodes are processed in *degree-sorted* order (sorted
by local in-degree desc, then A-window count desc) in tiles of 128; tile t
needs rounds = max in-tile degree, and one dma_gather fetches up to 8 rounds
(1024 rows) in a single instruction, issued round-robin over 4 SWDGE queues
(queues generate descriptors in parallel on the Q7 cores: ~2ns/row aggregate
vs ~8.6ns/row on one queue).  A DVE min-tree folds the gathered rounds.

dma_gather indices are int16, so the 50064-row padded table is addressed
through two overlapping windows (A = rows [0, 32768), B = rows
[17296, 50064)); each destination's edge list is split by source row at
MID=25032 and the two sub-lists occupy separate rounds.  Slots beyond a
destination's count in a window gather a dedicated +3.0 pad row (min-neutral:
table values are in [-1, 1]).

BN trickery: BN1's scale/shift is folded into Wf/bf (weights are adjusted on
device after a tiny AllReduce of the batch moments).  BN2 is applied *after*
aggregation: the table stores sign(gamma2) * tanh(...), so
min(a2*t + b2) == |a2| * min(sign(a2)*t) + b2, and |a2|/b2 are folded into
Wc1/bc1.  This keeps the BN2 AllReduce completely off the critical path.

Everything runs in a transposed activation layout ([features on partitions,
nodes on free]) so matmuls contract over the partition dim natively; the two
places that need node-major data (the h2 table, the aggregated features) use
PE transposes.
"""

import math
import numpy as np
import ml_dtypes

import concourse.bass as bass
import concourse.tile as tile
from concourse import mybir, library_config
from concourse.bass_utils import run_bass_kernel_spmd
from concourse.library_overlay import lower_extended_insts
from concourse.masks import make_identity
from concourse.tile import add_dep_helper

F32 = mybir.dt.float32
F32R = mybir.dt.float32r
BF16 = mybir.dt.bfloat16
I16 = mybir.dt.int16

# problem dims (hardcoded per contract)
C = 8
N = 50000
NS = N // C           # 6250 nodes per core
IN = 1024
STR = 768
H = 256
H2 = 2 * H            # 512
OUT = 64
EPS = 1e-5

KI = IN // 128        # 8
KS = STR // 128       # 6
HC = H // 128         # 2
K2 = H2 // 128        # 4

FT = 512              # free-dim node tile for phases A/B
NT = (NS + 127) // 128   # 49 node tiles for the aggregation phase
PAD = NT * 128           # 6272

PADR = 8              # +3.0 pad rows appended to each core's table block
NSP = NS + PADR       # 6258 rows per core in the gathered table
NROWS_G = C * NSP     # 50064
WIN = 32768           # int16 index window size
WINB_BASE = NROWS_G - WIN   # 17296
MID = (WINB_BASE + WIN) // 2  # 25032: src rows < MID go to window A
A_PAD = NS                          # core 0's first pad row (abs 6250)
B_PAD = (C - 1) * NSP + NS - WINB_BASE  # core 7's pad row, rebased (32760)
PADBIG = 3.0

CH = 8                # rounds per dma_gather (<= 1024 idxs / 128)
NQ = 4                # SWDGE queues

VE = 25               # packed small-vector columns
LINEARIZE = False


def _r(ap):
    return ap.bitcast(F32R)


def _col_tiles(n, t):
    out = []
    o = 0
    while o < n:
        out.append((o, min(t, n - o)))
        o += t
    return out


def build_program(schedule, totc):
    """Build the SPMD Bass program.  `schedule` is a list (len NT) of
    (rA_t, rB_t) round counts; identical on every core.  `totc` is the total
    number of int16 index columns.

    Wait-budget discipline: a self-loading fp32r Matmult can carry at most ONE
    sync wait in codegen, i.e. it may depend on at most one "proc" (engine /
    DMA lane) whose semaphore tick the PE has not already observed.  So every
    tensor a matmul reads is last-written by ACT (phases A/B) and DMA waits
    are absorbed by PE nops (pinned before their matmul group with non-sync
    edges).  Phase C reductions run on DVE; a per-group PE nop observes the
    DVE tick before the transposes/classifier matmuls run.
    """
    nc = bass.Bass(num_swdge_queues=NQ)
    AF = mybir.ActivationFunctionType

    xT = nc.declare_dram_parameter("xT", [IN, NS], F32R, isOutput=False)
    xsT = nc.declare_dram_parameter("xsT", [STR, NS], F32R, isOutput=False)
    idxd = nc.declare_dram_parameter("idx", [128, totc], I16, isOutput=False)
    wsem = nc.declare_dram_parameter("wsem", [IN, H], F32R, isOutput=False)
    wstr = nc.declare_dram_parameter("wstr", [STR, H], F32R, isOutput=False)
    wf = nc.declare_dram_parameter("wf", [H2, H], F32R, isOutput=False)
    wc1 = nc.declare_dram_parameter("wc1", [H, H], F32R, isOutput=False)
    wc2 = nc.declare_dram_parameter("wc2", [H, OUT], F32R, isOutput=False)
    vecs = nc.declare_dram_parameter("vecs", [128, VE], F32, isOutput=False)
    padc = nc.declare_dram_parameter("padc", [PADR, H], BF16, isOutput=False)
    outT = nc.declare_dram_parameter("outT", [OUT, PAD], F32, isOutput=True)

    table_local = nc.dram_tensor("table_local", [NSP, H], BF16)
    table = nc.dram_tensor("table", [C * NSP, H], BF16, addr_space="Shared")
    bn1_in = nc.dram_tensor("bn1_in", [128, 8], F32)
    bn1_out = nc.dram_tensor("bn1_out", [128, 8], F32, addr_space="Shared")
    bn2_in = nc.dram_tensor("bn2_in", [128, 4], F32)
    bn2_out = nc.dram_tensor("bn2_out", [128, 4], F32, addr_space="Shared")

    RG = [list(range(C))]
    ntiles = _col_tiles(NS, FT)
    n_ft = len(ntiles)

    with tile.TileContext(nc, linearize=LINEARIZE) as tc:
        touch_state = {}

        def pe_touch(ap):
            """Tiny matmul reading `ap` so the PE's vector clock observes the
            producer's semaphore tick via a REAL data dep (a 1-wait
            instruction); later matmuls reading the same producer then carry
            no extra wait.  Output goes to one persistent write-only psum
            (same tile every time -> same-engine WAW, no slot-release sems)."""
            if "pt" not in touch_state:
                ptile = touch_state["pool"].tile([1, 1], F32, tag="touch")
                touch_state["pt"] = ptile
            apf = ap.bitcast(F32) if ap.dtype == F32R else ap
            mm = nc.tensor.matmul(touch_state["pt"][:], apf, apf,
                                  start=True, stop=True)
            return mm

        def pin_after(mm, nop):
            if nop is not None:
                add_dep_helper(mm.ins, nop.ins, sync=False, reason="pe-order")

        with (
            tc.tile_pool(name="const", bufs=1) as cp,
            tc.tile_pool(name="psA", bufs=3, space="PSUM") as psA,
            tc.tile_pool(name="psT", bufs=2, space="PSUM") as psT,
            tc.tile_pool(name="psV", bufs=2, space="PSUM") as psV,
            tc.tile_pool(name="tp", bufs=1, space="PSUM") as tpool,
        ):
            touch_state["pool"] = tpool
            nc.gpsimd.load_library(library_config.mlp)
            # ---- constants ----
            ident = cp.tile([128, 128], F32, tag="ident")
            make_identity(nc, ident[:])
            ws_sb = cp.tile([128, KI, H], F32R, tag="ws")
            nc.sync.dma_start(out=ws_sb[:], in_=wsem[:].rearrange("(k p) h -> p k h", p=128))
            wsr_sb = cp.tile([128, KS, H], F32R, tag="wsr")
            nc.sync.dma_start(out=wsr_sb[:], in_=wstr[:].rearrange("(k p) h -> p k h", p=128))
            wf_sb = cp.tile([128, K2, H], F32R, tag="wfs")
            nc.sync.dma_start(out=wf_sb[:], in_=wf[:].rearrange("(k p) h -> p k h", p=128))
            wc1_sb = cp.tile([128, HC, H], F32R, tag="wc1s")
            nc.sync.dma_start(out=wc1_sb[:], in_=wc1[:].rearrange("(k p) h -> p k h", p=128))
            wc2_sb = cp.tile([128, HC, OUT], F32R, tag="wc2s")
            nc.sync.dma_start(out=wc2_sb[:], in_=wc2[:].rearrange("(k p) o -> p k o", p=128))
            vec_sb = cp.tile([128, VE], F32, tag="vecs")
            nc.sync.dma_start(out=vec_sb[:], in_=vecs[:])
            # table pad rows (min-neutral +3.0), written once
            pad_sb = cp.tile([PADR, H], BF16, tag="padsb")
            nc.sync.dma_start(out=pad_sb[:], in_=padc[:])
            nc.sync.dma_start(out=table_local[NS:NSP, :], in_=pad_sb[:])
            pe_touch(ident[:, 0:1])
            pe_touch(ws_sb[:, 0, 0:1])
            pe_touch(wsr_sb[:, 0, 0:1])
            pe_touch(wf_sb[:, 0, 0:1])
            pe_touch(wc1_sb[:, 0, 0:1])
            pe_touch(wc2_sb[:, 0, 0:1])
            # ACT / DVE observe the vec DMA lane once, so later bias/scale
            # reads never add a DMA wait to compute instructions.
            vtouch = cp.tile([128, 1], F32, tag="vt")
            vtouch2 = cp.tile([128, 1], F32, tag="vt2")
            nc.scalar.activation(out=vtouch[:], in_=vec_sb[:, 0:1], func=AF.Copy)
            nc.vector.tensor_scalar_mul(out=vtouch2[:], in0=vec_sb[:, 0:1],
                                        scalar1=1.0)

            # packed columns
            b_sem = vec_sb[:, 0:2]
            b_str = vec_sb[:, 2:4]
            gam1 = vec_sb[:, 4:8]
            bet1 = vec_sb[:, 8:12]
            bf_c = vec_sb[:, 12:14]
            gam2 = vec_sb[:, 14:16]
            bet2 = vec_sb[:, 16:18]
            bc1_c = vec_sb[:, 18:20]
            sflip = vec_sb[:, 20:22]
            bc2_c = vec_sb[:, 22:23]
            eps_c = vec_sb[:, 23:24]

            sums1 = cp.tile([128, K2, n_ft], F32, tag="sums1")
            sqs1 = cp.tile([128, K2, n_ft], F32, tag="sqs1")
            sums2 = cp.tile([128, HC, n_ft], F32, tag="sums2")
            sqs2 = cp.tile([128, HC, n_ft], F32, tag="sqs2")
            biasF = cp.tile([128, HC], F32, tag="biasF")
            bias1 = cp.tile([128, HC], F32, tag="bias1")

            # ================= phase A: refiners =================
            with (
                tc.tile_pool(name="hp", bufs=1) as hp,
                tc.tile_pool(name="xp", bufs=2) as xp,
                tc.tile_pool(name="t2p", bufs=4) as t2p,
                tc.tile_pool(name="asmp", bufs=3) as asmp,
            ):
                hT = hp.tile([128, K2, NS], F32R, tag="hT")

                def refiner(src_ap, w_sb, nk, bias_c, fc0, n0, nsz, nti, nop):
                    for hc in range(HC):
                        ps = psA.tile([128, nsz], F32, tag="mm")
                        for k in range(nk):
                            mm = nc.tensor.matmul(
                                ps[:], w_sb[:, k, hc * 128:(hc + 1) * 128],
                                src_ap[:, k, :], start=(k == 0), stop=(k == nk - 1))
                            if k == 0:
                                pin_after(mm, nop)
                        lin = t2p.tile([128, nsz], F32, tag="lk0")
                        nc.scalar.activation(out=lin[:], in_=ps[:], func=AF.Identity,
                                             bias=bias_c[:, hc:hc + 1], scale=1.0)
                        tmp = t2p.tile([128, nsz], F32, tag="lk1")
                        nc.scalar.mul(out=tmp[:], in_=lin[:], mul=0.01)
                        lk2 = t2p.tile([128, nsz], F32, tag="lk2")
                        nc.vector.tensor_tensor(out=lk2[:], in0=lin[:], in1=tmp[:],
                                                op=mybir.AluOpType.max)
                        hdst = hT[:, fc0 + hc, n0:n0 + nsz]
                        nc.scalar.activation(out=hdst, in_=lk2[:], func=AF.Identity,
                                             bias=0.0, scale=1.0)
                        nc.vector.tensor_reduce(
                            out=sums1[:, fc0 + hc, nti:nti + 1], in_=lk2[:],
                            op=mybir.AluOpType.add, axis=mybir.AxisListType.X)
                        sq = t2p.tile([128, nsz], F32, tag="sq")
                        nc.scalar.activation(out=sq[:], in_=lk2[:], func=AF.Square)
                        nc.vector.tensor_reduce(
                            out=sqs1[:, fc0 + hc, nti:nti + 1], in_=sq[:],
                            op=mybir.AluOpType.add, axis=mybir.AxisListType.X)

                for nti, (n0, nsz) in enumerate(ntiles):
                    xk = xp.tile([128, KI, nsz], F32R, tag="xin")
                    nc.sync.dma_start(
                        out=xk[:],
                        in_=xT[:].rearrange("(k p) n -> p k n", p=128)[:, :, n0:n0 + nsz])
                    nopx = pe_touch(xk[:, 0, 0:1])
                    refiner(xk, ws_sb, KI, b_sem, 0, n0, nsz, nti, nopx)
                    xs = xp.tile([128, KS, nsz], F32R, tag="xin")
                    nc.sync.dma_start(
                        out=xs[:],
                        in_=xsT[:].rearrange("(k p) n -> p k n", p=128)[:, :, n0:n0 + nsz])
                    nops = pe_touch(xs[:, 0, 0:1])
                    refiner(xs, wsr_sb, KS, b_str, HC, n0, nsz, nti, nops)

                # ---- BN1 moments -> AllReduce -> fold into Wf ----
                pay1 = cp.tile([128, 8], F32, tag="pay1")
                for fc in range(K2):
                    nc.vector.tensor_reduce(
                        out=pay1[:, fc:fc + 1], in_=sums1[:, fc, :],
                        op=mybir.AluOpType.add, axis=mybir.AxisListType.X)
                    nc.vector.tensor_reduce(
                        out=pay1[:, 4 + fc:5 + fc], in_=sqs1[:, fc, :],
                        op=mybir.AluOpType.add, axis=mybir.AxisListType.X)
                nc.gpsimd.dma_start(out=bn1_in[:], in_=pay1[:])
                nc.gpsimd.collective_compute(
                    "AllReduce", mybir.AluOpType.add, ins=[bn1_in[:]], outs=[bn1_out[:]],
                    replica_groups=RG)
                red1 = cp.tile([128, 8], F32, tag="red1")
                nc.gpsimd.dma_start(out=red1[:], in_=bn1_out[:])
                mg = cp.tile([128, K2], F32, tag="mg1")
                a1 = cp.tile([128, K2], F32, tag="a1")
                b1f = cp.tile([128, K2], F32, tag="b1f")
                b1 = cp.tile([128, K2], F32R, tag="b1")
                nc.vector.tensor_scalar_mul(out=mg[:], in0=red1[:, 0:4],
                                            scalar1=1.0 / (C * NS))
                nc.vector.tensor_scalar_mul(out=a1[:], in0=red1[:, 4:8],
                                            scalar1=1.0 / (C * NS))
                nc.vector.tensor_tensor(out=b1f[:], in0=mg[:], in1=mg[:],
                                        op=mybir.AluOpType.mult)
                nc.vector.tensor_tensor(out=a1[:], in0=a1[:], in1=b1f[:],
                                        op=mybir.AluOpType.subtract)
                nc.scalar.activation(out=a1[:], in_=a1[:], func=AF.Sqrt,
                                     bias=eps_c, scale=1.0)
                nc.vector.reciprocal(out=a1[:], in_=a1[:])
                nc.vector.tensor_tensor(out=a1[:], in0=a1[:], in1=gam1,
                                        op=mybir.AluOpType.mult)
                nc.vector.tensor_tensor(out=b1f[:], in0=mg[:], in1=a1[:],
                                        op=mybir.AluOpType.mult)
                nc.vector.tensor_tensor(out=b1f[:], in0=bet1, in1=b1f[:],
                                        op=mybir.AluOpType.subtract)
                nc.scalar.activation(out=b1[:], in_=b1f[:], func=AF.Identity)
                # biasF = b1 @ Wf + bf (original Wf), then scale Wf rows by a1
                for hc in range(HC):
                    pv = psV.tile([128, 1], F32, tag="v")
                    for k in range(K2):
                        nc.tensor.matmul(pv[:],
                                         wf_sb[:, k, hc * 128:(hc + 1) * 128].bitcast(F32),
                                         b1[:, k:k + 1].bitcast(F32), start=(k == 0),
                                         stop=(k == K2 - 1))
                    nc.scalar.activation(out=biasF[:, hc:hc + 1], in_=pv[:],
                                         func=AF.Identity,
                                         bias=bf_c[:, hc:hc + 1], scale=1.0)
                for k in range(K2):
                    nc.scalar.activation(out=wf_sb[:, k, :],
                                         in_=wf_sb[:, k, :].bitcast(F32),
                                         func=AF.Identity, bias=0.0,
                                         scale=a1[:, k:k + 1])

                # ================= phase B: fusion + table =================
                for nti, (n0, nsz) in enumerate(ntiles):
                    t2s = []
                    for hc in range(HC):
                        ps = psA.tile([128, nsz], F32, tag="mm")
                        for k in range(K2):
                            nc.tensor.matmul(
                                ps[:], wf_sb[:, k, hc * 128:(hc + 1) * 128],
                                hT[:, k, n0:n0 + nsz], start=(k == 0),
                                stop=(k == K2 - 1))
                        t2 = t2p.tile([128, nsz], F32, tag="t2")
                        nc.scalar.activation(out=t2[:], in_=ps[:], func=AF.Tanh,
                                             bias=biasF[:, hc:hc + 1], scale=1.0)
                        nc.vector.tensor_reduce(
                            out=sums2[:, hc, nti:nti + 1], in_=t2[:],
                            op=mybir.AluOpType.add, axis=mybir.AxisListType.X)
                        sq = t2p.tile([128, nsz], F32, tag="sq")
                        nc.scalar.activation(out=sq[:], in_=t2[:], func=AF.Square)
                        nc.vector.tensor_reduce(
                            out=sqs2[:, hc, nti:nti + 1], in_=sq[:],
                            op=mybir.AluOpType.add, axis=mybir.AxisListType.X)
                        ts = t2p.tile([128, nsz], F32, tag="t2s")
                        nc.scalar.activation(out=ts[:], in_=t2[:], func=AF.Identity,
                                             bias=0.0, scale=sflip[:, hc:hc + 1])
                        t2s.append(ts)
                    for nb in range((nsz + 127) // 128):
                        bsz = min(128, nsz - nb * 128)
                        asm = asmp.tile([128, HC, 128], BF16, tag="asm")
                        for hc in range(HC):
                            pt = psT.tile([128, 128], F32, tag="tr")
                            nc.tensor.transpose(
                                pt[:bsz, :], t2s[hc][:, nb * 128:nb * 128 + bsz], ident[:])
                            nc.scalar.activation(out=asm[:bsz, hc, :],
                                                 in_=pt[:bsz, :], func=AF.Copy)
                        r0 = n0 + nb * 128
                        nc.sync.dma_start(
                            out=table_local[r0:r0 + bsz, :].rearrange(
                                "n (a b) -> n a b", a=HC),
                            in_=asm[:bsz, :, :])

            # ---- collectives: table AllGather + BN2 AllReduce ----
            nc.gpsimd.collective_compute(
                "AllGather", mybir.AluOpType.bypass, ins=[table_local[:]],
                outs=[table[:]], replica_groups=RG)

            pay2 = cp.tile([128, 4], F32, tag="pay2")
            for hc in range(HC):
                nc.vector.tensor_reduce(
                    out=pay2[:, hc:hc + 1], in_=sums2[:, hc, :],
                    op=mybir.AluOpType.add, axis=mybir.AxisListType.X)
                nc.vector.tensor_reduce(
                    out=pay2[:, 2 + hc:3 + hc], in_=sqs2[:, hc, :],
                    op=mybir.AluOpType.add, axis=mybir.AxisListType.X)
            nc.gpsimd.dma_start(out=bn2_in[:], in_=pay2[:])
            nc.gpsimd.collective_compute(
                "AllReduce", mybir.AluOpType.add, ins=[bn2_in[:]], outs=[bn2_out[:]],
                replica_groups=RG)
            red2 = cp.tile([128, 4], F32, tag="red2")
            nc.gpsimd.dma_start(out=red2[:], in_=bn2_out[:])
            mg2 = cp.tile([128, HC], F32, tag="mg2")
            a2 = cp.tile([128, HC], F32, tag="a2")   # gamma2*rstd (signed)
            b2f = cp.tile([128, HC], F32, tag="b2f")
            b2 = cp.tile([128, HC], F32R, tag="b2")
            nc.vector.tensor_scalar_mul(out=mg2[:], in0=red2[:, 0:2],
                                        scalar1=1.0 / (C * NS))
            nc.vector.tensor_scalar_mul(out=a2[:], in0=red2[:, 2:4],
                                        scalar1=1.0 / (C * NS))
            nc.vector.tensor_tensor(out=b2f[:], in0=mg2[:], in1=mg2[:],
                                    op=mybir.AluOpType.mult)
            nc.vector.tensor_tensor(out=a2[:], in0=a2[:], in1=b2f[:],
                                    op=mybir.AluOpType.subtract)
            nc.scalar.activation(out=a2[:], in_=a2[:], func=AF.Sqrt,
                                 bias=eps_c, scale=1.0)
            nc.vector.reciprocal(out=a2[:], in_=a2[:])
            nc.vector.tensor_tensor(out=a2[:], in0=a2[:], in1=gam2,
                                    op=mybir.AluOpType.mult)
            nc.vector.tensor_tensor(out=b2f[:], in0=mg2[:], in1=a2[:],
                                    op=mybir.AluOpType.mult)
            nc.vector.tensor_tensor(out=b2f[:], in0=bet2, in1=b2f[:],
                                    op=mybir.AluOpType.subtract)
            nc.scalar.activation(out=b2[:], in_=b2f[:], func=AF.Identity)
            # bias1 = b2 @ Wc1 + bc1 (original Wc1); then Wc1 rows *= |a2|
            for hc in range(HC):
                pv = psV.tile([128, 1], F32, tag="v")
                for k in range(HC):
                    nc.tensor.matmul(pv[:],
                                     wc1_sb[:, k, hc * 128:(hc + 1) * 128].bitcast(F32),
                                     b2[:, k:k + 1].bitcast(F32), start=(k == 0),
                                     stop=(k == HC - 1))
                nc.scalar.activation(out=bias1[:, hc:hc + 1], in_=pv[:],
                                     func=AF.Identity,
                                     bias=bc1_c[:, hc:hc + 1], scale=1.0)
            a2a = cp.tile([128, HC], F32, tag="a2a")
            nc.vector.tensor_scalar_mul(out=a2a[:], in0=a2[:], scalar1=-1.0)
            nc.vector.tensor_tensor(out=a2a[:], in0=a2a[:], in1=a2[:],
                                    op=mybir.AluOpType.max)
            for k in range(HC):
                nc.scalar.activation(out=wc1_sb[:, k, :],
                                     in_=wc1_sb[:, k, :].bitcast(F32),
                                     func=AF.Identity, bias=0.0,
                                     scale=a2a[:, k:k + 1])

            # ================= phase C: gather-min + classifier =================
            with (
                tc.tile_pool(name="idxp", bufs=1) as idxp,
                tc.tile_pool(name="gbp", bufs=4) as gbp,
                tc.tile_pool(name="accp", bufs=6) as accp,
                tc.tile_pool(name="aggp", bufs=2) as aggp,
                tc.tile_pool(name="r1p", bufs=2) as r1p,
                tc.tile_pool(name="otp", bufs=3) as otp,
            ):
                idx_sb = idxp.tile([128, totc], I16, tag="idx")
                idma = nc.gpsimd.dma_start(out=idx_sb[:], in_=idxd[:])
                ni_regs = {}

                def ni_reg(v):
                    if v not in ni_regs:
                        ni_regs[v] = nc.gpsimd.to_reg(v)
                    return ni_regs[v]
                # absorb the conservative block-entry PE wait Tile emits on
                # the first PE instruction after the phase-B pools close
                # (anchored in this region via a dep on the idx DMA)
                c_nop = nc.tensor.nop()
                add_dep_helper(c_nop.ins, idma.ins, sync=True, reason="anchor")

                tabA = table[0:WIN, :]
                tabB = table[WINB_BASE:NROWS_G, :]
                col = 0
                qrr = 0

                GRP = 4
                for g0 in range(0, NT, GRP):
                    tl = list(range(g0, min(g0 + GRP, NT)))
                    gsz = len(tl) * 128
                    aggT = aggp.tile([128, HC, gsz], F32R, tag="aggT")
                    accs = []
                    for ti, t in enumerate(tl):
                        rA, rB = schedule[t]
                        rt = rA + rB
                        acc = accp.tile([128, H], F32, tag="acc")
                        if rt == 0:
                            nc.vector.memset(acc[:], PADBIG)
                            accs.append(acc)
                            continue
                        buf = gbp.tile([128, max(rt, 2), H], BF16, tag="gbuf")
                        s = 0
                        for win_ap, rw in ((tabA, rA), (tabB, rB)):
                            k0 = 0
                            while k0 < rw:
                                csz = min(CH, rw - k0)
                                ni = csz * 128
                                nc.gpsimd.dma_gather(
                                    buf[:, s:s + csz, :], win_ap,
                                    idx_sb[:, col:col + csz * 8],
                                    ni, ni_reg(ni), H, queue_num=qrr % NQ)
                                qrr += 1
                                col += csz * 8
                                k0 += csz
                                s += csz
                        # DVE min-tree over the rt gathered rounds; the last
                        # fold emits the fp32 acc
                        r = rt
                        while r > 2:
                            h = r // 2
                            nc.vector.tensor_tensor(
                                out=buf[:, 0:h, :], in0=buf[:, 0:h, :],
                                in1=buf[:, r - h:r, :], op=mybir.AluOpType.min)
                            r = r - h
                        if r == 2:
                            nc.vector.tensor_tensor(
                                out=acc[:], in0=buf[:, 0, :], in1=buf[:, 1, :],
                                op=mybir.AluOpType.min)
                        else:
                            nc.scalar.activation(out=acc[:], in_=buf[:, 0, :],
                                                 func=AF.Copy)
                        accs.append(acc)
                    gnop = None
                    for a in accs:
                        gnop = pe_touch(a[:, 0:1])
                        if g0 == 0:
                            add_dep_helper(gnop.ins, c_nop.ins, sync=False,
                                           reason="pe-order")
                    for ti, t in enumerate(tl):
                        for fc in range(HC):
                            pt = psT.tile([128, 128], F32, tag="tr")
                            tr = nc.tensor.transpose(
                                pt[:], accs[ti][:, fc * 128:(fc + 1) * 128], ident[:])
                            pin_after(tr, gnop)
                            nc.scalar.activation(
                                out=aggT[:, fc, ti * 128:(ti + 1) * 128], in_=pt[:],
                                func=AF.Copy)
                    r1 = r1p.tile([128, HC, gsz], F32R, tag="r1")
                    for hc in range(HC):
                        ps = psA.tile([128, gsz], F32, tag="mm")
                        for k in range(HC):
                            mm = nc.tensor.matmul(
                                ps[:], wc1_sb[:, k, hc * 128:(hc + 1) * 128],
                                aggT[:, k, :], start=(k == 0), stop=(k == HC - 1))
                            if k == 0:
                                pin_after(mm, gnop)
                        nc.scalar.activation(out=r1[:, hc, :], in_=ps[:], func=AF.Relu,
                                             bias=bias1[:, hc:hc + 1], scale=1.0)
                    ps2 = psA.tile([64, gsz], F32, tag="mm")
                    for k in range(HC):
                        nc.tensor.matmul(ps2[:], wc2_sb[:, k, :], r1[:, k, :],
                                         start=(k == 0), stop=(k == HC - 1))
                    ot = otp.tile([64, gsz], F32, tag="ot")
                    nc.scalar.activation(out=ot[:], in_=ps2[:], func=AF.Identity,
                                         bias=bc2_c[:64, :], scale=1.0)
                    nc.sync.dma_start(out=outT[:, g0 * 128:g0 * 128 + gsz], in_=ot[:])

    return nc


def _split_excess_waits(nc, budget=1):
    """Walrus codegen in this container rejects instructions carrying more
    than one sync wait.  Move excess waits onto standalone EventSemaphore
    instructions inserted immediately before the offender on the same
    engine queue (the same mechanism Tile's own barriers use)."""
    n = 0
    for f in nc.m.functions:
        for bb in f.blocks:
            out = []
            for ins in bb.instructions:
                si = ins.sync_info
                waits = list(si.on_wait) if si and si.on_wait else []
                if len(waits) > budget:
                    for w in waits[:-budget]:
                        ev = mybir.InstEventSemaphore(
                            name=f"evw-{n}", ins=[], outs=[])
                        n += 1
                        ev.engine = ins.engine
                        ev.sync_info = mybir.SyncInfo(on_wait=[w], on_update=[])
                        out.append(ev)
                    si.on_wait = waits[-budget:]
                out.append(ins)
            bb.instructions = out
    return n


# ---------------------------------------------------------------------------
# host side
# ---------------------------------------------------------------------------

def _prep(edge_index):
    """Shard edges by destination, split each destination's edge list into the
    two int16 index windows, degree-sort nodes per shard, and build the shared
    gather schedule plus per-core int16 index tables (wrapped in the
    16-partition dma_gather layout).

    Window membership: rows < WINB_BASE can only live in window A, rows >=
    WIN only in window B, rows in [WINB_BASE, WIN) are flexible.  Nodes are
    sorted by (degree desc, forced-A desc, forced-B desc) so per-128-dst-tile
    round budgets rA/rB stay close to the max degree; flexible edges are then
    assigned to satisfy (cA <= rA, d - cA <= rB) per node."""
    src = np.asarray(edge_index[0], dtype=np.int64)
    dst = np.asarray(edge_index[1], dtype=np.int64)
    owner = dst // NS
    dloc = (dst - owner * NS).astype(np.int64)
    sowner = src // NS
    # table rows are in natural per-core order: abs row of src
    sabs = sowner * NSP + (src - sowner * NS)
    fA = sabs < WINB_BASE           # forced window A
    fB = sabs >= WIN                # forced window B

    deg = np.zeros((C, NS), np.int64)
    aF = np.zeros((C, NS), np.int64)
    bF = np.zeros((C, NS), np.int64)
    perm = np.zeros((C, NS), np.int64)
    rank = np.zeros((C, NS), np.int64)
    for r in range(C):
        m = owner == r
        deg[r] = np.bincount(dloc[m], minlength=NS)
        aF[r] = np.bincount(dloc[m][fA[m]], minlength=NS)
        bF[r] = np.bincount(dloc[m][fB[m]], minlength=NS)
        perm[r] = np.lexsort((-bF[r], -aF[r], -deg[r]))
        rank[r][perm[r]] = np.arange(NS)

    # per-tile round budgets, max'd across cores so the program is SPMD-shared
    rA_t = np.zeros(NT, np.int64)
    rB_t = np.zeros(NT, np.int64)
    for r in range(C):
        aFs = aF[r][perm[r]]
        bFs = bF[r][perm[r]]
        degs = deg[r][perm[r]]
        for t in range(NT):
            i0, i1 = t * 128, min((t + 1) * 128, NS)
            rA_t[t] = max(rA_t[t], aFs[i0:i1].max())
    for r in range(C):
        bFs = bF[r][perm[r]]
        degs = deg[r][perm[r]]
        for t in range(NT):
            i0, i1 = t * 128, min((t + 1) * 128, NS)
            need_b = np.maximum(bFs[i0:i1], degs[i0:i1] - rA_t[t]).max()
            rB_t[t] = max(rB_t[t], need_b)
    schedule = [(int(rA_t[t]), int(rB_t[t])) for t in range(NT)]
    rAmax = int(rA_t.max()) if NT else 0
    rBmax = int(rB_t.max()) if NT else 0
    # total int16 columns (each round is 128 idxs = 8 wrapped columns)
    totc = int((rA_t.sum() + rB_t.sum()) * 8)

    # per-node cA: forced-A plus enough flexible edges that cB fits rB
    rB_of_rank = np.repeat(rB_t, 128)[:NS]
    idx16 = np.zeros((C, 128, totc), np.int16)
    for r in range(C):
        m = owner == r
        er = rank[r][dloc[m]]
        ab = sabs[m]
        # window of each edge: forced A, forced B, or flexible
        w_forced_b = fB[m]
        w_flex = (~fA[m]) & (~fB[m])
        cA_rank = np.maximum(aF[r][perm[r]],
                             deg[r][perm[r]] - rB_of_rank)  # per-rank target
        # order edges per destination: forced-A, flexible, forced-B
        wkey = np.where(fA[m], 0, np.where(w_flex, 1, 2))
        order = np.lexsort((wkey, er))
        er_s = er[order]
        ab_s = ab[order]
        cnt = np.bincount(er_s, minlength=NS)
        cum = np.concatenate([[0], np.cumsum(cnt)])
        within = np.arange(len(er_s)) - cum[er_s]
        # first cA_rank[er] edges (in forced-A/flex/forced-B order) -> A
        in_a = within < cA_rank[er_s]
        MA = np.full((PAD, max(rAmax, 1)), A_PAD, np.int64)
        MB = np.full((PAD, max(rBmax, 1)), B_PAD, np.int64)
        MA[er_s[in_a], within[in_a]] = ab_s[in_a]
        nb = ~in_a
        MB[er_s[nb], within[nb] - cA_rank[er_s[nb]]] = ab_s[nb] - WINB_BASE
        cols = []
        for t in range(NT):
            i0 = t * 128
            for M, rw in ((MA, int(rA_t[t])), (MB, int(rB_t[t]))):
                blk = M[i0:i0 + 128, :rw].T  # [rw, 128] rounds-first
                k0 = 0
                while k0 < rw:
                    csz = min(CH, rw - k0)
                    v = blk[k0:k0 + csz].reshape(-1)      # csz*128
                    w = v.reshape(-1, 16).T               # [16, csz*8]
                    cols.append(np.tile(w, (8, 1)))
                    k0 += csz
        idx16[r] = np.concatenate(cols, axis=1).astype(np.int16)

    return deg, perm, schedule, totc, idx16


_CACHE = {}


def kernel(**inputs):
    x = np.asarray(inputs["x"], np.float32)
    xs = np.asarray(inputs["x_struct"], np.float32)
    ei = np.asarray(inputs["edge_index"])
    W_sem = np.asarray(inputs["W_sem"], np.float32)
    b_sem = np.asarray(inputs["b_sem"], np.float32)
    W_str = np.asarray(inputs["W_str"], np.float32)
    b_str = np.asarray(inputs["b_str"], np.float32)
    g1 = np.asarray(inputs["bn1_gamma"], np.float32)
    be1 = np.asarray(inputs["bn1_beta"], np.float32)
    Wf = np.asarray(inputs["Wf"], np.float32)
    bf = np.asarray(inputs["bf"], np.float32)
    g2 = np.asarray(inputs["bn2_gamma"], np.float32)
    be2 = np.asarray(inputs["bn2_beta"], np.float32)
    Wc1 = np.asarray(inputs["Wc1"], np.float32)
    bc1 = np.asarray(inputs["bc1"], np.float32)
    Wc2 = np.asarray(inputs["Wc2"], np.float32)
    bc2 = np.asarray(inputs["bc2"], np.float32)

    deg, perm, schedule, totc, idx16 = _prep(ei)

    key = tuple(schedule)
    if key not in _CACHE:
        prog = build_program(schedule, totc)
        lower_extended_insts(prog)
        _split_excess_waits(prog)
        _CACHE[key] = prog
    nc = _CACHE[key]

    def pk2(v):   # [2*128] -> [128, 2] chunk-major
        return np.ascontiguousarray(v.reshape(-1, 128).T)

    vecs = np.zeros((128, VE), np.float32)
    vecs[:, 0:2] = pk2(b_sem)
    vecs[:, 2:4] = pk2(b_str)
    vecs[:, 4:8] = pk2(g1)
    vecs[:, 8:12] = pk2(be1)
    vecs[:, 12:14] = pk2(bf)
    vecs[:, 14:16] = pk2(g2)
    vecs[:, 16:18] = pk2(be2)
    vecs[:, 18:20] = pk2(bc1)
    vecs[:, 20:22] = pk2(np.where(g2 >= 0, 1.0, -1.0).astype(np.float32))
    vecs[:OUT, 22] = bc2
    vecs[:, 23] = EPS

    padc = np.full((PADR, H), PADBIG, ml_dtypes.bfloat16)

    in_maps = []
    for r in range(C):
        sl = slice(r * NS, (r + 1) * NS)
        in_maps.append({
            "xT": np.ascontiguousarray(x[sl].T),
            "xsT": np.ascontiguousarray(xs[sl].T),
            "idx": idx16[r],
            "wsem": W_sem, "wstr": W_str, "wf": Wf, "wc1": Wc1, "wc2": Wc2,
            "vecs": vecs, "padc": padc,
        })

    global _last_in_maps
    _last_in_maps = in_maps
    res = run_bass_kernel_spmd(nc, in_maps, list(range(C)))
    out = np.empty((N, OUT), np.float32)
    for r in range(C):
        oT = res.results[r]["outT"]
        out[r * NS + perm[r]] = oT[:, :NS].T

    # nodes with no incoming edges: reference yields relu(bc1) @ Wc2 + bc2
    # deg is indexed [core, local]; global id = core*NS + local
    empty = np.where(deg.reshape(-1) == 0)[0]
    if len(empty):
        const_row = np.maximum(bc1, 0.0) @ Wc2 + bc2
        out[empty] = const_row.astype(np.float32)
    return out
